# revision 20
# baseline (speedup 1.0000x reference)
"""DRMGCN (dual-branch 5-layer GCN + channel attention + outer product) on
8 TRN2 NeuronCores.

Strategy
--------
- Graph aggregation is cast as a dense matmul against the normalized
  adjacency (random graph => no usable block sparsity): agg = A_hat @ z,
  with A_hat built on host (self-loops + symmetric normalization),
  stored transposed (A_hat.T, src-major) in bf16. Contraction rows are
  padded to 10112 (79*128); dst columns to 10240 (8*1280).
- Nodes are sharded 8-way (1280/core). Each layer: local z = h @ W,
  AllGather z across cores, then aggT_shard = z_full.T-contract against
  the core's A_hat.T column slice on the tensor engine.
- h is kept TRANSPOSED [f, nodes] on-chip, making every matmul in the
  chain (z, agg, conv, final) transpose-free.
- Software pipelining: branches alternate (1 then 0) each layer, and
  each branch's z(i+1)+AllGather is emitted immediately after its own
  agg(i), so the collective always flies under the OTHER branch's
  ~100us agg matmul stream; per-branch DMA pools (7-deep) let the next
  agg's adjacency chunks prefetch across the branch switch.
- All constants are pre-transposed on host to [P, ...] layouts so each
  loads with a single contiguous line per partition (the strided loads
  were descriptor-bound and delayed the first AllGather).
- Channel attention folds AFTER the conv: conv is computed per-channel
  (right after each layer, hidden in the stream) into SBUF; the tiny
  5->25->5 MLP runs per-branch on-device, and channel outputs combine
  with att weights on the vector engine (relu(att*X)==att*X since X>=0,
  att>0). Branch 1 finishes one agg early, so its attention AllReduce
  hides under branch 0's last agg.
- Final [10000,128] @ [128,10000]: m-outer loop assembles full-width
  [128, 10240] bf16 rows in SBUF and writes them with one DMA each
  (128 x 20KB lines) -- the r-outer per-tile form was descriptor-bound.
- bf16 for all heavy matmul operands (fp32 matmul is 4x slower on PE;
  fp8 fails the 2e-2 tolerance: measured 3.9e-2); fp32 accumulation in
  PSUM; fp32 bias/activations.
"""

import numpy as np
import ml_dtypes

import concourse.mybir as mybir
import concourse.tile as tile
from concourse import bacc
from concourse.bass_utils import run_bass_kernel_spmd

NC = 8           # cores
N_NODE = 10000   # real nodes per branch
NPAD = 10240     # padded dst (multiple of 8*128)
SRCP = 10112     # padded src / contraction (multiple of 128)
S = NPAD // NC   # 1280 nodes per core
P = 128
SM = S // P      # 10 m-tiles per shard
F = 256          # feature dim
FC = F // P      # 2 feature chunks
L = 5            # gcn layers
OC = 128         # conv out channels
KC = SRCP // P   # 79 contraction chunks
NT = [(0, 512), (512, 512), (1024, 256)]  # n-tiles within a 1280 shard

F32 = mybir.dt.float32
BF16 = mybir.dt.bfloat16
BF = ml_dtypes.bfloat16
AF = mybir.ActivationFunctionType
ALU = mybir.AluOpType
RG = [list(range(NC))]

_CACHE = {}


def _build():
    nc = bacc.Bacc("TRN2", target_bir_lowering=False, debug=False, num_devices=NC)

    at_d, x0t_d, w_d, bt_d, cwt_d, cb_d = [], [], [], [], [], []
    fc1wt_d, fc1b_d, fc2wt_d, fc2b_d = [], [], [], []
    for br in range(2):
        at_d.append(nc.dram_tensor(f"at{br}", [KC, P, S], BF16, kind="ExternalInput"))
        # full (unsharded) x0, transposed: layer-0 z is computed redundantly
        # on every core so no AllGather is needed before the first agg
        x0t_d.append(nc.dram_tensor(f"x0t{br}", [P, FC, NPAD], BF16, kind="ExternalInput"))
        w_d.append(nc.dram_tensor(f"w{br}", [P, L, FC, F], BF16, kind="ExternalInput"))
        bt_d.append(nc.dram_tensor(f"bt{br}", [P, L, FC], F32, kind="ExternalInput"))
        cwt_d.append(nc.dram_tensor(f"cwt{br}", [P, L, FC, OC], BF16, kind="ExternalInput"))
        cb_d.append(nc.dram_tensor(f"cb{br}", [P, 1], F32, kind="ExternalInput"))
        fc1wt_d.append(nc.dram_tensor(f"fc1wt{br}", [L, 5 * L], F32, kind="ExternalInput"))
        fc1b_d.append(nc.dram_tensor(f"fc1b{br}", [5 * L, 1], F32, kind="ExternalInput"))
        fc2wt_d.append(nc.dram_tensor(f"fc2wt{br}", [5 * L, L], F32, kind="ExternalInput"))
        fc2b_d.append(nc.dram_tensor(f"fc2b{br}", [L, 1], F32, kind="ExternalInput"))
    out_d = nc.dram_tensor("out", [S, NPAD], BF16, kind="ExternalOutput")

    with tile.TileContext(nc) as tc:
        with (
            tc.tile_pool(name="const", bufs=1) as const,
            tc.tile_pool(name="big", bufs=2) as bigp,
            tc.tile_pool(name="htp", bufs=3) as htp,
            tc.tile_pool(name="sb", bufs=2) as sb,
            tc.tile_pool(name="zsb", bufs=2) as zsb,
            tc.tile_pool(name="zk0", bufs=7) as zkp0,
            tc.tile_pool(name="zk1", bufs=7) as zkp1,
            tc.tile_pool(name="at0", bufs=8) as atp0,
            tc.tile_pool(name="at1", bufs=8) as atp1,
            tc.tile_pool(name="psa", bufs=6, space="PSUM") as psa,
            tc.tile_pool(name="psz", bufs=2, space="PSUM") as psz,
            tc.tile_pool(name="dram", bufs=2, space="DRAM") as dram,
        ):
            zkp = [zkp0, zkp1]
            atp = [atp0, atp1]

            # ---- consts first (contiguous layouts drain the queues fast);
            #      branch-1 z-path tensors lead so z0(1) starts earliest ----
            w_sb = [None, None]
            for br in (1, 0):
                w_t = const.tile([P, L, FC, F], BF16, name=f"w_sb{br}")
                nc.sync.dma_start(w_t[:], w_d[br][:])
                w_sb[br] = w_t

            mx_sb = const.tile([P, 2 * L], F32, name="mx_sb")
            nc.vector.memset(mx_sb[:], 0.0)
            ones_sb = const.tile([1, P], F32, name="ones_sb")
            nc.vector.memset(ones_sb[:], 1.0)

            bt_sb, cwt_sb, cb_sb = [], [], []
            fc1wt_sb, fc1b_sb, fc2wt_sb, fc2b_sb = [], [], [], []
            for br in range(2):
                cw_t = const.tile([P, L, FC, OC], BF16, name=f"cwt_sb{br}")
                nc.sync.dma_start(cw_t[:], cwt_d[br][:])
                bt_t = const.tile([P, L, FC], F32, name=f"bt_sb{br}")
                nc.sync.dma_start(bt_t[:], bt_d[br][:])
                cb_t = const.tile([P, 1], F32, name=f"cb_sb{br}")
                nc.sync.dma_start(cb_t[:], cb_d[br][:])
                f1w = const.tile([L, 5 * L], F32, name=f"fc1wt_sb{br}")
                nc.sync.dma_start(f1w[:], fc1wt_d[br][:])
                f1b = const.tile([5 * L, 1], F32, name=f"fc1b_sb{br}")
                nc.sync.dma_start(f1b[:], fc1b_d[br][:])
                f2w = const.tile([5 * L, L], F32, name=f"fc2wt_sb{br}")
                nc.sync.dma_start(f2w[:], fc2wt_d[br][:])
                f2b = const.tile([L, 1], F32, name=f"fc2b_sb{br}")
                nc.sync.dma_start(f2b[:], fc2b_d[br][:])
                bt_sb.append(bt_t); cwt_sb.append(cw_t); cb_sb.append(cb_t)
                fc1wt_sb.append(f1w); fc1b_sb.append(f1b)
                fc2wt_sb.append(f2w); fc2b_sb.append(f2b)

            ht = [[None] * L, [None] * L]
            cc = [[None] * L, [None] * L]
            zf = [None, None]

            def emit_z_ag(br, i, hsrc):
                # z_shard = h_shard @ W[i] -> [S, F]; AllGather -> zf [NPAD, F]
                z_sb = zsb.tile([P, SM, F], BF16, name="z_sb")
                for m in range(SM):
                    zp = psz.tile([P, F], F32, name="zp", tag="psz")
                    for fc in range(FC):
                        nc.tensor.matmul(
                            zp[:],
                            hsrc[:, fc, m * P:(m + 1) * P],
                            w_sb[br][:, i, fc, :],
                            start=(fc == 0),
                            stop=(fc == FC - 1),
                        )
                    nc.vector.tensor_copy(z_sb[:, m, :], zp[:])
                zb = dram.tile([S, F], BF16, name="zb")
                nc.sync.dma_start(zb.rearrange("(m p) f -> p m f", p=P), z_sb[:])
                zft = dram.tile([NPAD, F], BF16, name="zf", addr_space="Shared")
                nc.gpsimd.collective_compute(
                    "AllGather", ALU.bypass,
                    replica_groups=RG, ins=[zb.opt()], outs=[zft.opt()],
                )
                zf[br] = zft

            def emit_z0_local(br):
                # z0_full = x0_full @ W[0], computed redundantly per core
                # (1.3 GFLOP) -> zf [NPAD, F] local DRAM; no collective.
                zft = dram.tile([NPAD, F], BF16, name="zf")
                Q = NPAD // 4
                for q in range(4):  # quarter-granularity loads pipeline better
                    x0h = bigp.tile([P, FC, Q], BF16, name="x0h", tag="big")
                    nc.sync.dma_start(x0h[:], x0t_d[br][:, :, q * Q:(q + 1) * Q])
                    for blk in range(2):  # 2 blocks of SM=10 m-tiles/quarter
                        z_sb = zsb.tile([P, SM, F], BF16, name="z_sb")
                        for m in range(SM):
                            zp = psz.tile([P, F], F32, name="zp", tag="psz")
                            for fc in range(FC):
                                nc.tensor.matmul(
                                    zp[:],
                                    x0h[:, fc, (blk * SM + m) * P:
                                        (blk * SM + m + 1) * P],
                                    w_sb[br][:, 0, fc, :],
                                    start=(fc == 0),
                                    stop=(fc == FC - 1),
                                )
                            nc.vector.tensor_copy(z_sb[:, m, :], zp[:])
                        row = q * Q + blk * S
                        nc.sync.dma_start(
                            zft[row:row + S, :].rearrange("(m p) f -> p m f", p=P),
                            z_sb[:],
                        )
                zf[br] = zft

            emit_z0_local(1)
            emit_z0_local(0)

            def emit_agg(br, i):
                # aggT_shard = relu((A_hat @ z_full).T slice + b); also mx col
                h_t = htp.tile([P, FC, S], BF16, name="h_t", tag="ht")
                ht[br][i] = h_t
                aps = [[psa.tile([P, 512], F32, name="aps", tag="psa")
                        for _ in NT] for _ in range(FC)]
                for k in range(KC):
                    zk = zkp[br].tile([P, F], BF16, name=f"zk{br}", tag="zk")
                    nc.sync.dma_start(zk[:], zf[br][k * P:(k + 1) * P, :])
                    atk = atp[br].tile([P, S], BF16, name=f"atk{br}", tag="atk")
                    nc.sync.dma_start(atk[:], at_d[br][k])
                    for fc in range(FC):
                        for n, (off, sz) in enumerate(NT):
                            nc.tensor.matmul(
                                aps[fc][n][:, :sz],
                                zk[:, fc * P:(fc + 1) * P],
                                atk[:, off:off + sz],
                                start=(k == 0),
                                stop=(k == KC - 1),
                            )
                for fc in range(FC):
                    for n, (off, sz) in enumerate(NT):
                        nc.scalar.activation(
                            h_t[:, fc, off:off + sz], aps[fc][n][:, :sz],
                            AF.Relu, bias=bt_sb[br][:, i, fc:fc + 1],
                        )
                nc.vector.reduce_max(
                    mx_sb[:, br * L + i: br * L + i + 1], h_t[:],
                    axis=mybir.AxisListType.XY,
                )

            def emit_conv_c(br, c):
                # per-channel conv: cc[br][c][oc, n] = sum_f cw[f, oc] * h[f, n]
                o_t = const.tile([P, S], BF16, name=f"cc{br}_{c}")
                cc[br][c] = o_t
                for n, (off, sz) in enumerate(NT):
                    cp = psz.tile([P, 512], F32, name="cp", tag="psz")
                    for fc in range(FC):
                        nc.tensor.matmul(
                            cp[:, :sz], cwt_sb[br][:, c, fc, :],
                            ht[br][c][:, fc, off:off + sz],
                            start=(fc == 0),
                            stop=(fc == FC - 1),
                        )
                    nc.vector.tensor_copy(o_t[:, off:off + sz], cp[:, :sz])

            def emit_att_pre(br):
                # partition-max AllReduce + transpose bounce: no PE ops here
                mxb = dram.tile([P, L], F32, name="mxb")
                nc.sync.dma_start(mxb[:], mx_sb[:, br * L:(br + 1) * L])
                mxr = dram.tile([P, L], F32, name="mxr", addr_space="Shared")
                nc.gpsimd.collective_compute(
                    "AllReduce", ALU.max,
                    replica_groups=RG, ins=[mxb.opt()], outs=[mxr.opt()],
                )
                mrow = sb.tile([1, L, P], F32, name="mrow", bufs=1)
                nc.sync.dma_start(mrow[:], mxr.rearrange("p i -> () i p"))
                att0 = sb.tile([1, L], F32, name="att0")
                nc.vector.reduce_max(att0[:], mrow[:], axis=mybir.AxisListType.X)
                a0d = dram.tile([1, L], F32, name="a0d")
                nc.sync.dma_start(a0d[:], att0[:])
                a0col = sb.tile([L, 1], F32, name="a0col")
                nc.sync.dma_start(a0col[:], a0d.rearrange("() c -> c ()"))
                return a0col

            def emit_att_mlp(br, a0col):
                # tiny 5->25->5 MLP + broadcast att across partitions
                p1 = psz.tile([5 * L, 1], F32, name="p1", tag="psz")
                nc.tensor.matmul(p1[:], fc1wt_sb[br][:], a0col[:], start=True, stop=True)
                y1 = sb.tile([5 * L, 1], F32, name="y1")
                nc.scalar.activation(y1[:], p1[:], AF.Relu, bias=fc1b_sb[br][:])
                p2 = psz.tile([L, 1], F32, name="p2", tag="psz")
                nc.tensor.matmul(p2[:], fc2wt_sb[br][:], y1[:], start=True, stop=True)
                attc = sb.tile([L, 1], F32, name="attc")
                nc.scalar.activation(attc[:], p2[:], AF.Sigmoid, bias=fc2b_sb[br][:])
                ar_d = dram.tile([1, L], F32, name="ar_d")
                nc.sync.dma_start(ar_d.rearrange("() c -> c ()"), attc[:])
                attrow = sb.tile([1, L], F32, name="attrow")
                nc.sync.dma_start(attrow[:], ar_d[:])
                pb = psz.tile([P, L], F32, name="pb", tag="psz")
                nc.tensor.matmul(pb[:], ones_sb[:], attrow[:], start=True, stop=True)
                attb = sb.tile([P, L], F32, name="attb")
                nc.vector.tensor_copy(attb[:], pb[:])
                return attb

            def emit_combine(br, attb, o_bf):
                # o_bf[oc, n] = sum_c att[c] * cc[br][c][oc, n] + cb
                o32 = sb.tile([P, S], F32, name="o32", bufs=1)
                nc.vector.tensor_scalar_mul(o32[:], cc[br][0][:], attb[:, 0:1])
                for c in range(1, L):
                    nc.vector.scalar_tensor_tensor(
                        o32[:], cc[br][c][:], attb[:, c:c + 1], o32[:],
                        op0=ALU.mult, op1=ALU.add,
                    )
                nc.vector.tensor_scalar_add(o_bf[:], o32[:], cb_sb[br][:])

            # ---- main loop: branch 1 first so its tail hides under agg(0,4)
            for i in range(L):
                for br in (1, 0):
                    emit_agg(br, i)
                    if i < L - 1:
                        emit_z_ag(br, i + 1, ht[br][i])
                    emit_conv_c(br, i)

            # ---- attention + combine + final (branch 1 first) ----
            a0col1 = emit_att_pre(1)
            attb1 = emit_att_mlp(1, a0col1)
            oyt = sb.tile([P, S], BF16, name="oyt", bufs=1)
            emit_combine(1, attb1, oyt)
            oyb = dram.tile([P, S], BF16, name="oyb")
            nc.sync.dma_start(oyb[:], oyt[:])
            # AR0 goes on the CC queue BEFORE the oy AllGather: its input is
            # ready at relu(0,4) and branch-0's MLP shouldn't wait for the AG
            a0col0 = emit_att_pre(0)
            oyf = dram.tile([NC * P, S], BF16, name="oyf", addr_space="Shared")
            nc.gpsimd.collective_compute(
                "AllGather", ALU.bypass,
                replica_groups=RG, ins=[oyb.opt()], outs=[oyf.opt()],
            )
            ktall = const.tile([P, NC, S], BF16, name="ktall")
            nc.sync.dma_start(ktall[:], oyf.rearrange("(r p) s -> p r s", p=P))
            attb0 = emit_att_mlp(0, a0col0)
            oxt = sb.tile([P, S], BF16, name="oxt", bufs=1)
            emit_combine(0, attb0, oxt)

            # m-outer final: half-width [128, 5120] rows double-buffer through
            # the two big-pool slots -- finer pipelining than full-width rows
            H = NPAD // 2
            for m in range(SM):
                for half in range(2):
                    fo = bigp.tile([P, H], BF16, name="fo", tag="big")
                    for r in range(half * 4, half * 4 + 4):
                        for n, (off, sz) in enumerate(NT):
                            fps = psa.tile([P, 512], F32, name="fps", tag="psa")
                            nc.tensor.matmul(
                                fps[:, :sz], oxt[:, m * P:(m + 1) * P],
                                ktall[:, r, off:off + sz], start=True, stop=True,
                            )
                            nc.vector.tensor_copy(
                                fo[:, (r - half * 4) * S + off:
                                   (r - half * 4) * S + off + sz], fps[:, :sz]
                            )
                    nc.sync.dma_start(
                        out_d[m * P:(m + 1) * P, half * H:(half + 1) * H], fo[:]
                    )
    nc.compile()
    return nc


def _build_at(edges, ew):
    """Dense transposed normalized adjacency A_hat.T, [SRCP, NPAD]."""
    src = np.asarray(edges[0], dtype=np.int64)
    dst = np.asarray(edges[1], dtype=np.int64)
    w = np.asarray(ew, dtype=np.float64)
    deg = np.ones(N_NODE, dtype=np.float64)  # self loops, weight 1
    np.add.at(deg, dst, w)
    dinv = 1.0 / np.sqrt(deg)
    norm = (dinv[src] * w * dinv[dst]).astype(np.float32)
    at = np.zeros((SRCP, NPAD), dtype=np.float32)
    np.add.at(at, (src, dst), norm)
    ii = np.arange(N_NODE)
    at[ii, ii] += (dinv * dinv).astype(np.float32)
    return at


def _prep_branch(x, ew, W, b, cw, cb, f1w, f1b, f2w, f2b, edges):
    at = _build_at(edges, ew)
    xp = np.zeros((NPAD, F), dtype=np.float32)
    xp[:N_NODE] = np.asarray(x, dtype=np.float32)
    x0t = np.ascontiguousarray(xp.T).astype(BF)          # [F, NPAD]
    # w: [P, L, FC, F] so each core loads it as one contiguous line/partition
    wq = np.ascontiguousarray(
        np.asarray(W, np.float32).reshape(L, FC, P, F).transpose(2, 0, 1, 3)
    ).astype(BF)
    bt = np.ascontiguousarray(
        np.asarray(b, np.float32).reshape(L, FC, P).transpose(2, 0, 1)
    ).astype(np.float32)
    cwt = np.ascontiguousarray(
        np.asarray(cw, np.float32)[:, :, :, 0]           # [oc, c, f]
        .transpose(2, 1, 0)                               # [f, c, oc]
        .reshape(FC, P, L, OC)
        .transpose(1, 2, 0, 3)                            # [P, L, FC, OC]
    ).astype(BF)
    cbq = np.asarray(cb, np.float32).reshape(P, 1)
    f1wt = np.ascontiguousarray(np.asarray(f1w, np.float32).T)  # [5,25]
    f1bq = np.asarray(f1b, np.float32).reshape(5 * L, 1)
    f2wt = np.ascontiguousarray(np.asarray(f2w, np.float32).T)  # [25,5]
    f2bq = np.asarray(f2b, np.float32).reshape(L, 1)
    return at, x0t, wq, bt, cwt, cbq, f1wt, f1bq, f2wt, f2bq


def _make_in_maps(inputs):
    br0 = _prep_branch(
        inputs["x_m"], inputs["w_m"], inputs["Wx"], inputs["bx"],
        inputs["cnnx_w"], inputs["cnnx_b"], inputs["fc1x_w"], inputs["fc1x_b"],
        inputs["fc2x_w"], inputs["fc2x_b"], inputs["edges_m"],
    )
    br1 = _prep_branch(
        inputs["x_d"], inputs["w_d"], inputs["Wy"], inputs["by"],
        inputs["cnny_w"], inputs["cnny_b"], inputs["fc1y_w"], inputs["fc1y_b"],
        inputs["fc2y_w"], inputs["fc2y_b"], inputs["edges_d"],
    )

    in_maps = []
    for k in range(NC):
        m = {}
        for br, (at, x0t, wq, bt, cwt, cbq, f1wt, f1bq, f2wt, f2bq) in enumerate(
            (br0, br1)
        ):
            sl = slice(k * S, (k + 1) * S)
            m[f"at{br}"] = np.ascontiguousarray(at[:, sl]).astype(BF).reshape(KC, P, S)
            # x0t: full [P, FC, NPAD] (same on every core; layer-0 z is
            # computed redundantly so no AllGather precedes the first agg)
            m[f"x0t{br}"] = np.ascontiguousarray(
                x0t.reshape(FC, P, NPAD).transpose(1, 0, 2)
            )
            m[f"w{br}"] = wq
            m[f"bt{br}"] = bt
            m[f"cwt{br}"] = cwt
            m[f"cb{br}"] = cbq
            m[f"fc1wt{br}"] = f1wt
            m[f"fc1b{br}"] = f1bq
            m[f"fc2wt{br}"] = f2wt
            m[f"fc2b{br}"] = f2bq
        in_maps.append(m)
    return in_maps


def kernel(**inputs):
    if "nc" not in _CACHE:
        _CACHE["nc"] = _build()
    nc = _CACHE["nc"]
    in_maps = _make_in_maps(inputs)
    res = run_bass_kernel_spmd(nc, in_maps, core_ids=list(range(NC)))
    full = np.concatenate([res.results[k]["out"] for k in range(NC)], axis=0)
    return np.ascontiguousarray(full[:N_NODE, :N_NODE]).astype(np.float32)
